# revision 25
# baseline (speedup 1.0000x reference)
"""Trainium2 Bass kernel for the LIIF non-parametric per-pixel mini-MLP.

Reference computation (per branch, per pixel p = (b,h,w)):
    channels c of feat reshape to W[head, o, i] with c = head*64 + o*8 + i
    t[T, i] = t_coord[T]  (broadcast over i)
    h = einsum('OI,TI->TO', W0, t);  then for k in 1..3: h = W_k @ relu(h)
    out[T] = h[T, 0]

Algebraic identity: t enters rank-1 in T and relu(s*t) = relu(s)*relu(t) +
relu(-s)*relu(-t) (disjoint support), so every intermediate stays in
span{u, v} with u = relu(t), v = relu(-t):
    s0[i]  = sum_j W0[i, j]
    a1 = relu(s0),            b1 = relu(-s0)
    a2 = relu(W1 @ a1),       b2 = relu(W1 @ b1)
    a3 = relu(W2 @ a2),       b3 = relu(W2 @ b2)
    alpha = W3[0, :] . a3,    beta = W3[0, :] . b3
    out[T] = alpha * u[T] + beta * v[T]
Only channels 0:200 of 256 are needed (row 0 of W3).

On-chip mapping (per unit = 512 pixels x both branches; [partition, free]):
    F0/F1/F2 [128, 512] views of a big SBUF tensor (chunked bulk DMA):
        partitions 0:64 = x_real channel slots, 64:128 = x_imag;
        channel g*64 + p%64 at group g.
    X1 = CM1^T @ F0      (PE)   s0 replicated to all (o,i) slots, both branches
    P1a = max(X1,0)*F1   (DVE)  fused relu+mult
    P1b = min(X1,0)*F1   (DVE)  stored = -true_P1b
    X2a = CM1^T @ P1a, X2b = CM1^T @ P1b   (PE; X2b stored = -true)
    P2a = max(X2a,0)*F2, P2b = min(X2b,0)*F2  (DVE; P2b stored = -true)
    X3[32,512] = C3A^T @ P2a + C3BN^T @ P2b   (PE accum; C3BN = -C3B fixes sign)
    P3 = max(X3,0)*F34   (DVE)  F34 = W3row0 repeated [re,re,im,im]
    XO[128,512] = G2^T @ P3    (PE)  partitions = (branch, T), rank-2 expansion
    O = copy(XO) -> fp16 SBUF (ACT), per-unit DMA out.

All matmul operands are fp16 (PE runs 1 cycle/column vs 4 for fp32; fp16 has
8x finer mantissa than bf16 at identical cost and our values stay << 65504)
and all HBM traffic is fp16; PSUM stays fp32.

The PE instruction stream is software-pipelined 3 rounds deep
(X1(r+2), X2ab(r+1), X3ab(r), XO(r-1)) so no matmul ever waits on a DVE
result produced in the same round. A short burst of dummy warmup matmuls
on memset tiles keeps the PE busy while the first input chunk lands, so the
clock is already ramped when real columns start streaming.

Sharding: 8 cores, core k -> batch b = k//2, h-half = k%2 (64 h-rows each).
"""

import os

import numpy as np

import concourse.bass as bass
import concourse.bacc as bacc
import concourse.tile as tile
from concourse import mybir
from concourse import bass_utils

F32 = mybir.dt.float32
F16 = mybir.dt.float16
NPF16 = np.dtype(np.float16)

NUM_CORES = 8
H_SH = 64             # h rows per core
W_ = 128
T_ = 64
N_UNITS = 16          # units per core; each unit covers 4 h rows = 512 px
PX = 512              # pixels per unit
CHUNKS = [1, 1, 2, 4, 4, 4]       # units per input-DMA chunk (ramp-friendly)
CHUNK_START = [0, 1, 2, 4, 8, 12]
N_WARMUP = int(os.environ.get("KERNEL_WARMUP", "0"))


def _build_const_mats(t_coord: np.ndarray):
    """Host-side constant matrices (tiny, derived from fixed structure + t_coord)."""
    # M1[k = 8i+j, m = 8o+i] = 1 : rep-reduce within one branch block
    m1 = np.zeros((64, 64), np.float32)
    for o in range(8):
        for i in range(8):
            for j in range(8):
                m1[8 * i + j, 8 * o + i] = 1.0
    cm1 = np.zeros((128, 128), np.float32)
    cm1[0:64, 0:64] = m1
    cm1[64:128, 64:128] = m1

    # C3A/C3BN [128, 32]: reduce products to X3 rows [a_re, b_re, a_im, b_im].
    # C3BN carries a -1 so the sign-inverted b-stream (stored = -true) lands
    # with the correct sign in the accumulated X3.
    c3a = np.zeros((128, 32), np.float32)
    c3bn = np.zeros((128, 32), np.float32)
    for i in range(8):
        for j in range(8):
            c3a[8 * i + j, i] = 1.0              # a3_re from P2a re-half
            c3a[64 + 8 * i + j, 16 + i] = 1.0    # a3_im from P2a im-half
            c3bn[8 * i + j, 8 + i] = 1.0         # b3_re from P2b re-half
            c3bn[64 + 8 * i + j, 24 + i] = 1.0   # b3_im from P2b im-half

    # G2 [32, 128]: rank-2 expansion. row 8*(2*br + s) + i, col 64*br + T
    t = t_coord.astype(np.float32)
    u = np.maximum(t, 0.0)
    v = np.maximum(-t, 0.0)
    g2 = np.zeros((32, 128), np.float32)
    for br in range(2):
        for i in range(8):
            g2[8 * (2 * br + 0) + i, 64 * br:64 * (br + 1)] = u
            g2[8 * (2 * br + 1) + i, 64 * br:64 * (br + 1)] = v
    return cm1, c3a, c3bn, g2


def _build_program():
    nc = bacc.Bacc("TRN2", target_bir_lowering=False, debug=False,
                   enable_asserts=False)
    # xp[p, u, g, px] = x[br, g*64 + c, 4u + px//128, px%128], p = 64*br + c
    xp_d = nc.dram_tensor("xp", [128, N_UNITS, 3, PX], F16, kind="ExternalInput").ap()
    # xt[q, u, px]: q = 16*br_pair + 8*dup + c for channels 192:200, [re,re,im,im]
    xt_d = nc.dram_tensor("xt", [128, 4, PX], F16, kind="ExternalInput").ap()
    cmats_d = nc.dram_tensor("cmats", [128, 320], F16, kind="ExternalInput").ap()
    out_d = nc.dram_tensor("out", [128, N_UNITS, PX], F16, kind="ExternalOutput").ap()

    MAX_ = mybir.AluOpType.max
    MIN_ = mybir.AluOpType.min
    MULT = mybir.AluOpType.mult

    mm = nc.tensor.matmul
    stt = nc.vector.scalar_tensor_tensor
    RELU = mybir.ActivationFunctionType.Relu

    with tile.TileContext(nc) as tc:
        with (
            tc.tile_pool(name="consts", bufs=1) as consts,
            tc.tile_pool(name="xpool", bufs=1) as xpool,
            tc.tile_pool(name="ppool", bufs=2) as ppool,
            tc.tile_pool(name="apool", bufs=2) as apool,
            tc.tile_pool(name="opool", bufs=4) as opool,
            tc.tile_pool(name="psum", bufs=1, space="PSUM") as psum,
        ):
            # Warmup source: memset tile, no DMA dependency. The warmup
            # matmuls below keep the PE continuously busy during the input
            # DMA ramp so the clock is at full speed for the first real unit.
            WU = consts.tile([128, 128], F16)
            nc.vector.memset(WU, 0.0)

            CT = consts.tile([128, 320], F16)
            nc.sync.dma_start(out=CT, in_=cmats_d)
            CM1 = CT[:, 0:128]
            C3A = CT[:, 128:160]
            C3BN = CT[:, 160:192]
            G2S = [CT[32 * m:32 * m + 32, 192:320] for m in range(4)]

            # Unit 0's three groups ride in front as separate DMAs, in
            # consumption order (F0 for X1, then F1 for P1, then F2), so the
            # first rounds never wait on a bulk chunk.
            F0G = []
            for g in range(3):
                F0g = xpool.tile([128, PX], F16, tag=f"f0{g}", name=f"F0g{g}")
                nc.sync.dma_start(out=F0g, in_=xp_d[:, 0, g, :])
                F0G.append(F0g)

            xp_tiles = [None]
            # xt is small and needed from round 0 on; load it right after
            # unit 0 so P3 never stalls on it.
            XT = xpool.tile([128, 4, PX], F16, tag="xt")
            nc.sync.dma_start(out=XT, in_=xt_d)
            for c, (n, s) in enumerate(zip(CHUNKS, CHUNK_START)):
                if c == 0:
                    continue
                XPc = xpool.tile([128, n, 3, PX], F16, tag=f"xp{c}")
                nc.sync.dma_start(out=XPc, in_=xp_d[:, s:s + n])
                xp_tiles.append(XPc)

            def fview(u, g):
                if u == 0:
                    return F0G[g]
                for c, (n, s) in enumerate(zip(CHUNKS, CHUNK_START)):
                    if c > 0 and s <= u < s + n:
                        return xp_tiles[c][:, u - s, g, :]
                raise AssertionError(u)

            # ---- PE warmup burst (dummy matmuls, no data deps) ----
            # Rides the X1 psum banks (rotation continues into the real X1s)
            # so no extra PSUM is consumed.
            for w in range(N_WARMUP):
                XW = psum.tile([128, PX], F32, tag="X1", bufs=2, name="XW")
                mm(XW[:, 0:32], WU, WU[:, 0:32])

            live = {}

            def st_x1(u):
                X1 = psum.tile([128, PX], F32, tag="X1", bufs=2)
                mm(X1, CM1, fview(u, 0))
                live[("X1", u)] = X1

            def st_p1(u):
                # a-side fused on DVE; b-side relu on ACT (PSUM -> fp16 SBUF)
                # with the multiply on the otherwise-idle Pool engine. This
                # keeps DVE/round below the PE round during full-speed
                # phases. Positive storage: P1b = relu(-X1)*F1 = true value.
                X1 = live.pop(("X1", u))
                P1a = ppool.tile([128, PX], F16, tag="P1a", bufs=3)
                stt(out=P1a, in0=X1, scalar=0.0, in1=fview(u, 1), op0=MAX_, op1=MULT)
                live[("P1a", u)] = P1a
                A1n = apool.tile([128, PX], F16, tag="A1n", name="A1n", bufs=3)
                nc.scalar.activation(A1n, X1, RELU, scale=-1.0)
                live[("A1n", u)] = A1n

            def st_p1b(u):
                A1n = live.pop(("A1n", u))
                P1b = ppool.tile([128, PX], F16, tag="P1b", bufs=3)
                nc.gpsimd.tensor_tensor(out=P1b, in0=A1n, in1=fview(u, 1),
                                        op=MULT)
                live[("P1b", u)] = P1b

            def st_x2(u):
                P1a = live.pop(("P1a", u))
                P1b = live.pop(("P1b", u))
                X2a = psum.tile([128, PX], F32, tag="X2a", bufs=1)
                mm(X2a, CM1, P1a)
                X2b = psum.tile([128, PX], F32, tag="X2b", bufs=1)
                mm(X2b, CM1, P1b)
                live[("X2", u)] = (X2a, X2b)

            def st_p2(u):
                X2a, X2b = live.pop(("X2", u))
                P2a = ppool.tile([128, PX], F16, tag="P2a")
                stt(out=P2a, in0=X2a, scalar=0.0, in1=fview(u, 2), op0=MAX_, op1=MULT)
                P2b = ppool.tile([128, PX], F16, tag="P2b")
                stt(out=P2b, in0=X2b, scalar=0.0, in1=fview(u, 2), op0=MAX_, op1=MULT)
                live[("P2", u)] = (P2a, P2b)

            def st_x3(u):
                # Units 4q..4q+3 write disjoint partition quarters of one
                # quad PSUM tile (PE array column tiling), so P3 runs as a
                # single [128, 512] STT per 4 units instead of 4 small ones.
                q, m = divmod(u, 4)
                P2a, P2b = live.pop(("P2", u))
                if m == 0:
                    live[("X3Q", q)] = psum.tile([128, PX], F32, tag="X3Q",
                                                 bufs=2, name="X3Q")
                X3Q = live[("X3Q", q)]
                dst = X3Q[32 * m:32 * m + 32, :]
                mm(dst, C3A, P2a, start=True, stop=False,
                   tile_position=(0, 32 * m))
                mm(dst, C3BN, P2b, start=False, stop=True,
                   tile_position=(0, 32 * m))

            def st_p3q(q):
                X3Q = live.pop(("X3Q", q))
                P3Q = ppool.tile([128, PX], F16, tag="P3Q", name="P3Q")
                stt(out=P3Q, in0=X3Q, scalar=0.0, in1=XT[:, q, :],
                    op0=MAX_, op1=MULT)
                live[("P3Q", q)] = P3Q

            def st_p3h(q, h):
                # Half-group P3 (used for the last quad): lets the first two
                # XOs of the group start two rounds earlier, shrinking the
                # output drain tail.
                sl = slice(64 * h, 64 * h + 64)
                if h == 0:
                    live[("P3Q", q)] = ppool.tile([128, PX], F16, tag="P3Q",
                                                  name="P3Qh")
                X3Q = live[("X3Q", q)] if h == 0 else live.pop(("X3Q", q))
                stt(out=live[("P3Q", q)][sl, :], in0=X3Q[sl, :], scalar=0.0,
                    in1=XT[sl, q, :], op0=MAX_, op1=MULT)

            def st_xo(u):
                q, m = divmod(u, 4)
                P3Q = live[("P3Q", q)]
                if m == 3:
                    del live[("P3Q", q)]
                XO = psum.tile([128, PX], F32, tag="XO", bufs=2)
                mm(XO, G2S[m], P3Q[32 * m:32 * m + 32, :],
                   tile_position=(32 * m, 0))
                live[("XO", u)] = XO

            def st_out(u):
                XO = live.pop(("XO", u))
                O = opool.tile([128, PX], F16, tag="O", name="O")
                nc.scalar.copy(O, XO)
                nc.sync.dma_start(out=out_d[:, u], in_=O)

            # Software-pipelined rounds: PE never consumes a same-round DVE
            # product. Emit order inside a round keeps the three CM1 matmuls
            # adjacent (X1, X2a, X2b).
            N = N_UNITS
            for r in range(-4, N + 5):
                if 0 <= r + 4 < N:
                    st_x1(r + 4)
                if 0 <= r + 1 < N:
                    st_x2(r + 1)
                if 0 <= r < N:
                    st_x3(r)
                if 0 <= r - 4 < N - 4:
                    st_xo(r - 4)
                if 12 <= r - 2 < N:
                    st_xo(r - 2)
                if 0 <= r + 4 < N:
                    st_p1(r + 4)
                if 0 <= r + 3 < N:
                    st_p1b(r + 3)
                if 0 <= r + 1 < N:
                    st_p2(r + 1)
                if 0 <= r < N and r % 4 == 3 and r // 4 < 3:
                    st_p3q(r // 4)  # one quad STT; inputs landed this round
                if r == N - 3:
                    st_p3h(3, 0)   # units 12-13 ready two rounds early
                if r == N - 1:
                    st_p3h(3, 1)   # units 14-15
                if 0 <= r - 4 < N - 4:
                    st_out(r - 4)
                if 12 <= r - 2 < N:
                    st_out(r - 2)
    nc.compile()
    return nc


_PROGRAM_CACHE = {}


def _get_program():
    if "nc" not in _PROGRAM_CACHE:
        _PROGRAM_CACHE["nc"] = _build_program()
    return _PROGRAM_CACHE["nc"]


def _make_in_maps(x_real, x_imag, t_coord):
    cm1, c3a, c3bn, g2 = _build_const_mats(np.asarray(t_coord))
    cmats = np.zeros((128, 320), np.float32)
    cmats[:, 0:128] = cm1
    cmats[:, 128:160] = c3a
    cmats[:, 160:192] = c3bn
    cmats[:, 192:320] = np.tile(g2, (4, 1))
    cmats = cmats.astype(NPF16)
    x_real = np.asarray(x_real)
    x_imag = np.asarray(x_imag)
    in_maps = []
    for core in range(NUM_CORES):
        b = core // 2
        h0 = H_SH * (core % 2)
        xs = np.stack([
            x_real[b, 0:192, h0:h0 + H_SH, :],
            x_imag[b, 0:192, h0:h0 + H_SH, :],
        ])  # [2, 192, H, W]
        # xp[(br, c), u, g, px] = xs[br, g*64 + c, 4u + px//128, px%128]
        xp = np.ascontiguousarray(
            xs.reshape(2, 3, 64, N_UNITS, 4, W_)
            .transpose(0, 2, 3, 1, 4, 5)
            .reshape(128, N_UNITS, 3, PX)).astype(NPF16)
        x3r = x_real[b, 192:200, h0:h0 + H_SH, :]
        x3i = x_imag[b, 192:200, h0:h0 + H_SH, :]
        # xtq[32*m + r, q, px] = xt_rows[r, 4q + m, px]: F34 for units
        # 4q..4q+3 stacked on partition quarters, matching the X3 quad tile.
        xt = np.ascontiguousarray(
            np.stack([x3r, x3r, x3i, x3i])
            .reshape(32, 4, 4, PX).transpose(2, 0, 1, 3)
            .reshape(128, 4, PX)).astype(NPF16)
        in_maps.append({"xp": xp, "xt": xt, "cmats": cmats})
    return in_maps


def _assemble(results):
    out = np.empty((2, 4, T_, 128, W_), np.float32)
    for core in range(NUM_CORES):
        b = core // 2
        h0 = H_SH * (core % 2)
        # arr[64*br + T, u, ...] -> [br, T, h = 4u + j, w]
        arr = results[core]["out"].astype(np.float32)
        out[:, b, :, h0:h0 + H_SH, :] = arr.reshape(2, T_, H_SH, W_)
    return out


def kernel_with_info(x_real, x_imag, t_coord, trace=False):
    nc = _get_program()
    in_maps = _make_in_maps(x_real, x_imag, t_coord)
    res = bass_utils.run_bass_kernel_spmd(
        nc, in_maps, core_ids=list(range(NUM_CORES)), trace=trace)
    return _assemble(res.results), res


def kernel(x_real, x_imag, t_coord):
    out, _ = kernel_with_info(x_real, x_imag, t_coord)
    return out
